# revision 22
# baseline (speedup 1.0000x reference)
"""Trainium2 Bass kernel for nn_CausalSelfAttention_70832600646065 (v2).

Sliding-window causal GQA attention (B=2, T=2048, C=1024, NH=16, NKV=4,
HD=64, window=1024) with RoPE + RMSNorm on q/k, a value-embedding gate, and
an output projection.

Sharding: sequence-parallel over 8 cores. Core c handles batch c//4, query
rows [512*(c%4), 512*(c%4)+512) with a 1024-row key/value halo.

v2 changes vs v1 (248.7us):
  - All inputs host-packed into contiguous [128, N] tensors; priority-ordered
    DMA issue (wkv+xt first on the sync ring; wq/wp mid-phase-A on the
    scalar ring) so the first projection matmul starts ~3us in.
  - PE warmup burst at t=0 (HAM clock-gate release) covering the DMA lead-in.
  - All transposes (K, Q, Y) moved off the tensor engine onto the DMA XBAR
    (dma_start transpose=True on the HWDGE rings): kills 88 PE transposes
    and their PSUM->SBUF copies.
  - K/V projections fused into one 512-wide matmul chain per row tile.
  - Per-rt pipelined phase A (proj -> rope -> norm -> transpose per tile).
  - Attention: query heads permuted (host-side) so device pair p = heads
    (2p, 2p+1) sit in opposite PE row halves (GQA group parity) -> QK
    matmuls issued interleaved to disjoint 64-row groups run concurrently.
  - PSUM: 2x3-bank score strips + 2x1-bank AV accumulators (4 heads each)
    = 8 banks exactly; out-projection PSUM time-shares the accumulator
    slots via matching tags.
  - Softmax denominators via a validity 65th column of V (as v1); one
    batched reciprocal + one fused normalize multiply per 4 heads.
  - Output stored/DMA'd as bf16 (host casts to f32).
"""

import sys

if "/opt/trn_rl_repo" not in sys.path:
    sys.path.insert(0, "/opt/trn_rl_repo")

import numpy as np
import ml_dtypes

import concourse.bass as bass
import concourse.bacc as bacc
import concourse.mybir as mybir
import concourse.tile as tile
from concourse.bass_utils import run_bass_kernel_spmd
from concourse.masks import make_identity

F32 = mybir.dt.float32
BF16 = mybir.dt.bfloat16
AF = mybir.ActivationFunctionType
OP = mybir.AluOpType

B, T, C = 2, 2048, 1024
NH, NKV, HD = 16, 4, 64
VEC = 32
WIN = 1024
QR = 512           # query rows per core
KR = QR + WIN      # key rows per core (incl. halo)
NQT = QR // 128    # 4 query row tiles
NKT = KR // 128    # 12 key row tiles
NCT = C // 128     # 8 contraction tiles
NJB = WIN // 128 + 1  # 9 key tiles in any 128-row query tile's window
EPS = float(np.finfo(np.float32).eps)
N_CORES = 8

# device q-head order: pair p = (dh[2p], dh[2p+1]) pairs an even-group head
# with an odd-group head so the two heads of a pair use disjoint PE row halves
DH = [0, 4, 1, 5, 2, 6, 3, 7, 8, 12, 9, 13, 10, 14, 11, 15]
GDEV = [h // 4 for h in DH]          # kv group per device head


def _rope_stats(nc, pools, src_sb, cs_t, sc_t, rr_dst, ms_dst, nh):
    """src_sb: [128, nh*64] bf16 SBUF in two-major layout (x1 of all heads |
    x2 of all heads). rr_dst: same layout, rope output. ms_dst: [128, nh]
    fp32 receiving sum(rr^2) per head. cs_t: [128, 2, 32] (cos|sin) AP;
    sc_t: (sin|cos)."""
    hw = nh * 32
    tA = pools["rtmp"].tile([128, nh * HD], BF16, tag="tA", name="tA")
    tB = pools["rtmp"].tile([128, nh * HD], BF16, tag="tB", name="tB")
    sq = pools["rtmp"].tile([128, nh * HD], BF16, tag="sq", name="sq")
    s4 = src_sb.rearrange("p (two h d) -> p two h d", two=2, d=32)
    a4 = tA[:].rearrange("p (two h d) -> p two h d", two=2, d=32)
    b4 = tB[:].rearrange("p (two h d) -> p two h d", two=2, d=32)
    csb = cs_t.unsqueeze(2).broadcast_to([128, 2, nh, 32])
    scb = sc_t.unsqueeze(2).broadcast_to([128, 2, nh, 32])
    # tA = (x1*cos | x2*sin); tB = (x1*sin | x2*cos)  (split across engines)
    nc.gpsimd.tensor_tensor(a4, s4, csb, op=OP.mult)
    nc.vector.tensor_tensor(b4, s4, scb, op=OP.mult)
    # rr = (x1*cos + x2*sin | x2*cos - x1*sin)
    nc.vector.tensor_tensor(rr_dst[:, 0:hw], tA[:, 0:hw], tA[:, hw:2 * hw],
                            op=OP.add)
    nc.vector.tensor_tensor(rr_dst[:, hw:2 * hw], tB[:, hw:2 * hw],
                            tB[:, 0:hw], op=OP.subtract)
    nc.scalar.activation(sq[:], rr_dst, AF.Square)
    sq4 = sq[:].rearrange("p (two h d) -> p two h d", two=2, d=32)
    mtmp = pools["ms"].tile([128, 2 * nh], F32, tag="mtmp", name="mtmp")
    nc.vector.tensor_reduce(mtmp[:], sq4, axis=mybir.AxisListType.X, op=OP.add)
    m2 = mtmp[:].rearrange("p (two h) -> p two h", two=2)
    nc.gpsimd.tensor_tensor(ms_dst, m2[:, 0], m2[:, 1], op=OP.add)


def _rsqrt(nc, pools, ms, n, tag):
    """rsqrt(ms*(1/HD) + eps) over a [128, n] fp32 tile on DVE only."""
    nc.vector.tensor_scalar(ms, ms, 1.0 / HD, EPS, op0=OP.mult, op1=OP.add)
    sh = pools["ms"].tile([128, n], mybir.dt.int32, tag=f"sh{tag}",
                          name=f"sh{tag}")
    nc.vector.tensor_scalar(sh[:], ms.bitcast(mybir.dt.int32), 1, None,
                            op0=OP.logical_shift_right)
    nc.vector.tensor_scalar(sh[:], sh[:], -1, 0x5F3759DF, op0=OP.mult,
                            op1=OP.add)
    r0 = sh[:].bitcast(F32)
    t0 = pools["ms"].tile([128, n], F32, tag=f"t0{tag}", name=f"t0{tag}")
    for _ in range(2):
        nc.vector.tensor_tensor(t0[:], r0, r0, op=OP.mult)
        nc.vector.scalar_tensor_tensor(t0[:], ms, -0.5, t0[:],
                                       op0=OP.mult, op1=OP.mult)
        nc.vector.scalar_tensor_tensor(r0, t0[:], 1.5, r0,
                                       op0=OP.add, op1=OP.mult)
    return r0


def build_program():
    nc = bacc.Bacc("TRN2", target_bir_lowering=False, debug=False,
                   num_devices=N_CORES)

    wkv_d = nc.declare_dram_parameter("wkv", [128, NCT * 512], BF16,
                                      isOutput=False)
    xt_d = nc.declare_dram_parameter("xt", [128, NCT * KR], BF16,
                                     isOutput=False)
    aux_d = nc.declare_dram_parameter("aux", [128, NKT * 260], BF16,
                                      isOutput=False)
    csc_d = nc.declare_dram_parameter("csc", [128, NKT * 128], F32,
                                      isOutput=False)
    wq_d = nc.declare_dram_parameter("wq", [128, NCT * 1024], BF16,
                                     isOutput=False)
    wp_d = nc.declare_dram_parameter("wp", [128, NCT * 1024], BF16,
                                     isOutput=False)
    y_d = nc.declare_dram_parameter("y", [QR, C], BF16, isOutput=True)

    with tile.TileContext(nc) as tc:
        with (
            tc.tile_pool(name="wgt", bufs=1) as wgt,
            tc.tile_pool(name="persist", bufs=1) as persist,
            tc.tile_pool(name="small", bufs=1) as small,
        ):
            # ---- input DMAs, priority order on the sync ring ---------------
            wkv_all = wgt.tile([128, NCT, 512], BF16, tag="wkv", name="wkv_all")
            nc.sync.dma_start(wkv_all[:], wkv_d.ap().rearrange(
                "p (c k) -> p c k", c=NCT))
            xt_all = wgt.tile([128, NCT, KR], BF16, tag="xt", name="xt_all")
            nc.sync.dma_start(xt_all[:], xt_d.ap().rearrange(
                "p (c k) -> p c k", c=NCT))
            aux_all = wgt.tile([128, NKT, 260], BF16, tag="aux", name="aux_all")
            nc.sync.dma_start(aux_all[:], aux_d.ap().rearrange(
                "p (r k) -> p r k", r=NKT))
            csc_all = wgt.tile([128, NKT, 4, 32], F32, tag="csc",
                               name="csc_all")
            nc.sync.dma_start(csc_all[:], csc_d.ap().rearrange(
                "p (r two d) -> p r two d", r=NKT, two=4))
            wq_all = wgt.tile([128, NCT, 1024], BF16, tag="wq", name="wq_all")
            nc.sync.dma_start(wq_all[:], wq_d.ap().rearrange(
                "p (c k) -> p c k", c=NCT))
            wp_all = wgt.tile([128, NCT, 1024], BF16, tag="wp", name="wp_all")
            nc.sync.dma_start(wp_all[:], wp_d.ap().rearrange(
                "p (c k) -> p c k", c=NCT))

            xt_sb = [xt_all[:, ct] for ct in range(NCT)]
            wkv_sb = [wkv_all[:, ct] for ct in range(NCT)]
            wq_sb = [wq_all[:, ct] for ct in range(NCT)]
            wp_sb = [wp_all[:, ct] for ct in range(NCT)]
            cs_sb = [csc_all[:, rt, 0:2] for rt in range(NKT)]
            sc_sb = [csc_all[:, rt, 2:4] for rt in range(NKT)]

            ident = small.tile([128, 128], BF16, tag="ident")
            make_identity(nc, ident[:])
            epsb = small.tile([128, 1], F32, tag="epsb")
            nc.gpsimd.memset(epsb[:], EPS)
            # mask_lo: keep p >= f (window edge, jb==0)
            mask_lo = small.tile([128, 128], BF16, tag="mask_lo")
            nc.gpsimd.memset(mask_lo[:], 1.0)
            nc.gpsimd.affine_select(
                out=mask_lo[:], in_=mask_lo[:], compare_op=OP.is_ge, fill=0.0,
                base=0, pattern=[[-1, 128]], channel_multiplier=1,
            )
            # mask_hi: keep p <= f (causal diagonal, jb==8)
            mask_hi = small.tile([128, 128], BF16, tag="mask_hi")
            nc.gpsimd.memset(mask_hi[:], 1.0)
            nc.gpsimd.affine_select(
                out=mask_hi[:], in_=mask_hi[:], compare_op=OP.is_ge, fill=0.0,
                base=0, pattern=[[1, 128]], channel_multiplier=-1,
            )

            # persistent intermediates (single tiles; blocked along dim 1 so
            # one batched DMA-transpose per source tile fills all blocks)
            KT_all = persist.tile([128, NKV // 2, KR], BF16, tag="KT",
                                  name="KT_all")
            QT_all = persist.tile([128, NH // 2, QR], BF16, tag="QT",
                                  name="QT_all")
            Vv_sb = [persist.tile([128, NKV, HD + 1], BF16, tag=f"Vv{rt}",
                                  name=f"Vv{rt}") for rt in range(NKT)]
            YT_all = persist.tile([128, NCT, QR], BF16, tag="YT",
                                  name="YT_all")

            # ---- phase A ---------------------------------------------------
            with (
                tc.tile_pool(name="pkv", bufs=2, space="PSUM") as pkv,
                tc.tile_pool(name="pwarm", bufs=1, space="PSUM") as pwarm,
                tc.tile_pool(name="asb", bufs=3) as asb,
                tc.tile_pool(name="asm", bufs=3) as asm,
                tc.tile_pool(name="astat", bufs=1) as astat,
            ):
                pools = {"rtmp": asb, "ms": asm}

                # PE warmup: ~10us of dummy matmuls (no DMA deps) to release
                # the HAM clock gate while inputs stream in.
                wscr = small.tile([128, 512], BF16, tag="wscr", name="wscr")
                nc.gpsimd.memset(wscr[:], 0.0)
                warm = pwarm.tile([128, 512], F32, tag="warm", name="warm")
                for w in range(24):
                    nc.tensor.matmul(warm[:], ident[:], wscr[:],
                                     start=(w == 0), stop=(w == 23))

                ms_k = astat.tile([128, NKT * NKV], F32, tag="ms_k")
                ms_q = astat.tile([128, NQT * NH], F32, tag="ms_q")
                rrk_sb = [persist.tile([128, NKV * HD], BF16, tag=f"rrk{rt}",
                                       name=f"rrk{rt}") for rt in range(NKT)]
                rrq_sb = [persist.tile([128, NH * HD], BF16, tag=f"rrq{i}",
                                       name=f"rrq{i}") for i in range(NQT)]

                def emit_kv_tile(rt):
                    rs = slice(rt * 128, (rt + 1) * 128)
                    kvp = pkv.tile([128, 512], F32, tag="kvp", name="kvp")
                    for ct in range(NCT):
                        nc.tensor.matmul(kvp[:], xt_sb[ct][:, rs], wkv_sb[ct],
                                         start=(ct == 0), stop=(ct == NCT - 1))
                    ksb = asb.tile([128, NKV * HD], BF16, tag="ksb",
                                   name="ksb")
                    nc.scalar.copy(ksb[:], kvp[:, 0:256])
                    # V = vp + ve_gated (gate precomputed on host)
                    ve3 = aux_all[:, rt, 0:256].rearrange(
                        "p (h d) -> p h d", h=NKV)
                    vp3 = kvp[:, 256:512].rearrange("p (h d) -> p h d", h=NKV)
                    nc.vector.tensor_tensor(Vv_sb[rt][:, :, 0:HD], vp3, ve3,
                                            op=OP.add)
                    nc.gpsimd.tensor_copy(
                        Vv_sb[rt][:, :, HD:HD + 1],
                        aux_all[:, rt, 256:260].unsqueeze(2))
                    _rope_stats(nc, pools, ksb[:], cs_sb[rt], sc_sb[rt],
                                rrk_sb[rt][:], ms_k[:, rt * NKV:(rt + 1) * NKV],
                                NKV)
                    # rsqrt via ACT: exp(-0.5 * ln(ms/HD + eps)), 2ULP each
                    msl = ms_k[:, rt * NKV:(rt + 1) * NKV]
                    lk = asm.tile([128, NKV], F32, tag="lk", name="lk")
                    nc.scalar.activation(lk[:], msl, AF.Ln, scale=1.0 / HD,
                                         bias=epsb[:])
                    rk = asm.tile([128, NKV], F32, tag="rk", name="rk")
                    nc.scalar.activation(rk[:], lk[:], AF.Exp, scale=-0.5)
                    kn = asb.tile([128, NKV * HD], BF16, tag="kn", name="kn")
                    kn4 = kn[:].rearrange("p (h two d) -> p two h d",
                                          two=2, d=32)
                    rr4 = rrk_sb[rt][:].rearrange("p (two h d) -> p two h d",
                                                  two=2, d=32)
                    rkb = rk[:].unsqueeze(1).unsqueeze(3).broadcast_to(
                        [128, 2, NKV, 32])
                    nc.gpsimd.tensor_tensor(kn4, rr4, rkb, op=OP.mult)
                    nc.sync.dma_start(KT_all[:, :, rs], kn[:], transpose=True)

                def emit_q_tile(it):
                    rt = (WIN // 128) + it
                    rs = slice(rt * 128, (rt + 1) * 128)
                    qsb = asb.tile([128, NH * HD], BF16, tag="qsb", name="qsb")
                    for half in range(2):
                        qp = pkv.tile([128, 512], F32, tag="kvp", name="qp")
                        for ct in range(NCT):
                            nc.tensor.matmul(
                                qp[:], xt_sb[ct][:, rs],
                                wq_sb[ct][:, half * 512:(half + 1) * 512],
                                start=(ct == 0), stop=(ct == NCT - 1))
                        nc.scalar.copy(qsb[:, half * 512:(half + 1) * 512],
                                       qp[:])
                    _rope_stats(nc, pools, qsb[:], cs_sb[rt], sc_sb[rt],
                                rrq_sb[it][:], ms_q[:, it * NH:(it + 1) * NH],
                                NH)
                    msl = ms_q[:, it * NH:(it + 1) * NH]
                    lq = asm.tile([128, NH], F32, tag="lq", name="lq")
                    nc.scalar.activation(lq[:], msl, AF.Ln, scale=1.0 / HD,
                                         bias=epsb[:])
                    rq = asm.tile([128, NH], F32, tag="rq", name="rq")
                    nc.scalar.activation(rq[:], lq[:], AF.Exp, scale=-0.5)
                    qn = asb.tile([128, NH * HD], BF16, tag="qsb", name="qn")
                    qn4 = qn[:].rearrange("p (h two d) -> p two h d",
                                          two=2, d=32)
                    rr4 = rrq_sb[it][:].rearrange("p (two h d) -> p two h d",
                                                  two=2, d=32)
                    rqb = rq[:].unsqueeze(1).unsqueeze(3).broadcast_to(
                        [128, 2, NH, 32])
                    nc.gpsimd.tensor_tensor(qn4, rr4, rqb, op=OP.mult)
                    nc.sync.dma_start(QT_all[:, :, it * 128:(it + 1) * 128],
                                      qn[:], transpose=True)

                # KV halo tiles first, then interleave Q tiles (wq lands
                # after wkv+xt; qt0's attention needs KT 0-8 + QT it0)
                for rt in range(9):
                    emit_kv_tile(rt)
                emit_q_tile(0)
                emit_kv_tile(9)
                emit_q_tile(1)
                emit_kv_tile(10)
                emit_q_tile(2)
                emit_kv_tile(11)
                emit_q_tile(3)

            # ---- phase B: attention + fused output projection --------------
            with (
                tc.tile_pool(name="pst", bufs=1, space="PSUM") as pst,
                tc.tile_pool(name="pacc", bufs=1, space="PSUM") as pacc,
                tc.tile_pool(name="bpt", bufs=2) as bpt,
                tc.tile_pool(name="brc", bufs=4) as brc,
                tc.tile_pool(name="by", bufs=2) as by,
                tc.tile_pool(name="bob", bufs=2) as bob,
            ):
                ob_prev = None  # (qt, Y tile) awaiting out-projection

                def emit_oproj(qt_prev, half):
                    tag = "pa" if half == 0 else "pb"
                    pr = pacc.tile([128, 512], F32, tag=tag, name=f"pr{half}")
                    for ct in range(NCT):
                        nc.tensor.matmul(
                            pr[:],
                            YT_all[:, ct, qt_prev * 128:(qt_prev + 1) * 128],
                            wp_sb[ct][:, half * 512:(half + 1) * 512],
                            start=(ct == 0), stop=(ct == NCT - 1))
                    nc.vector.tensor_copy(
                        ob_prev[1][:, half * 512:(half + 1) * 512], pr[:])
                    if half == 1:
                        nc.sync.dma_start(
                            y_d.ap()[qt_prev * 128:(qt_prev + 1) * 128, :],
                            ob_prev[1][:])

                for qt in range(NQT):
                    its = slice(qt * 128, (qt + 1) * 128)
                    Y_t = by.tile([128, C], BF16, tag="Y", name="Yt")
                    acc = {}

                    def emit_av(p, pt2):
                        # AV for pair p into its batch accumulator; after the
                        # second pair of a batch: normalize + Y transposes.
                        batch, sub = divmod(p, 2)
                        tag = "pa" if (batch % 2) == 0 else "pb"
                        if sub == 0:
                            acc[batch] = pacc.tile([128, 2, 2, HD + 1], F32,
                                                   tag=tag, name=f"acc{batch}")
                        for hh in range(2):
                            g = GDEV[2 * p + hh]
                            for jb in range(NJB):
                                nc.tensor.matmul(
                                    acc[batch][:, sub, hh],
                                    pt2[:, hh, jb * 128:(jb + 1) * 128],
                                    Vv_sb[qt + jb][:, g, :],
                                    start=(jb == 0), stop=(jb == NJB - 1))
                        if sub == 1:
                            a = acc[batch]
                            rc = brc.tile([128, 4], F32, tag="rc", name="rc")
                            nc.vector.reciprocal(
                                rc[:],
                                a[:, :, :, HD].rearrange("p a b -> p (a b)"))
                            rcb = rc[:].rearrange("p (a b) -> p a b", a=2) \
                                .unsqueeze(3).broadcast_to([128, 2, 2, HD])
                            yv = Y_t[:, batch * 256:(batch + 1) * 256] \
                                .rearrange("p (a b d) -> p a b d", a=2, b=2)
                            nc.vector.tensor_tensor(yv, a[:, :, :, 0:HD], rcb,
                                                    op=OP.mult)
                            if batch == 3:
                                # one batched transpose for the whole Y tile
                                nc.sync.dma_start(YT_all[:, :, its], Y_t[:],
                                                  transpose=True)

                    prev = None  # (p, pt2) exp'd but AV not yet emitted
                    for p in range(8):
                        ktp = KT_all[:, p // 4]
                        # QK: heads (2p, 2p+1) interleaved to row halves;
                        # both strips live in one padded 6-bank PSUM tile
                        st2 = pst.tile([128, 2, NJB * 128], F32, tag="st",
                                       padded_shape=[128, 2, 1536],
                                       name="st2")
                        for jb in range(NJB):
                            jts = slice((qt + jb) * 128, (qt + jb + 1) * 128)
                            js = slice(jb * 128, (jb + 1) * 128)
                            nc.tensor.matmul(st2[:, 0, js], ktp[0:64, jts],
                                             QT_all[0:64, p, its],
                                             start=True, stop=True)
                            nc.tensor.matmul(st2[:, 1, js], ktp[64:128, jts],
                                             QT_all[64:128, p, its],
                                             start=True, stop=True)
                        # previous pair's AV fills the PE while this pair's
                        # exp runs on the scalar engine
                        if prev is not None:
                            emit_av(*prev)
                        # out-projection of the previous row tile fills the
                        # remaining PE gap (tag rotation: pa frees after
                        # normalize(batch0) ~p==3, pb after batch1 ~p==6)
                        if ob_prev is not None and p == 4:
                            emit_oproj(ob_prev[0], 0)
                        # softmax: one exp per pair (no max-subtraction
                        # needed: |s|/8 <= 8), masks batched over both heads
                        pt2 = bpt.tile([128, 2, NJB * 128], BF16, tag="pt",
                                       name="pt2")
                        nc.scalar.activation(pt2[:], st2[:], AF.Exp,
                                             scale=1.0 / np.sqrt(HD))
                        nc.vector.tensor_tensor(
                            pt2[:, :, 0:128], pt2[:, :, 0:128],
                            mask_lo[:].unsqueeze(1).broadcast_to([128, 2, 128]),
                            op=OP.mult)
                        nc.vector.tensor_tensor(
                            pt2[:, :, WIN:WIN + 128], pt2[:, :, WIN:WIN + 128],
                            mask_hi[:].unsqueeze(1).broadcast_to([128, 2, 128]),
                            op=OP.mult)
                        prev = (p, pt2)
                    emit_av(*prev)
                    if ob_prev is not None:
                        emit_oproj(ob_prev[0], 1)
                    ob_prev = (qt, bob.tile([128, C], BF16, tag="ob",
                                            name="ob"))
                emit_oproj(ob_prev[0], 0)
                emit_oproj(ob_prev[0], 1)
    nc.compile()
    return nc


_CACHED = {}


def _get_program():
    if "nc" not in _CACHED:
        _CACHED["nc"] = build_program()
    return _CACHED["nc"]


def _prep_inputs(x, ve, cos, sin, Wq, Wk, Wv, Wproj, Wgate):
    bf = ml_dtypes.bfloat16
    # K: natural head order, two-major per rope convention
    wk2 = Wk.reshape(C, NKV, 2, 32).transpose(0, 2, 1, 3).reshape(C, NKV * HD)
    # fused [Wk2 | Wv] then ct-blocked [128, NCT*512]
    wkv = np.concatenate([wk2, Wv], axis=1)  # [C, 512]
    wkv_p = np.ascontiguousarray(
        wkv.reshape(NCT, 128, 512).transpose(1, 0, 2).reshape(128, NCT * 512)
        .astype(bf))
    # Q: device head order DH, two-major
    wq2 = (Wq.reshape(C, NH, 2, 32)[:, DH].transpose(0, 2, 1, 3)
           .reshape(C, NH * HD))
    wq_p = np.ascontiguousarray(
        wq2.reshape(NCT, 128, 1024).transpose(1, 0, 2).reshape(128, NCT * 1024)
        .astype(bf))
    # Wproj rows permuted to device head order
    wp2 = Wproj.reshape(NH, HD, C)[DH].reshape(C, C)
    wp_p = np.ascontiguousarray(
        wp2.reshape(NCT, 128, 1024).transpose(1, 0, 2).reshape(128, NCT * 1024)
        .astype(bf))
    cos2 = cos[0, :, 0, :]
    sin2 = sin[0, :, 0, :]
    in_maps = []
    for c in range(N_CORES):
        b, j = divmod(c, N_CORES // B)
        q0 = QR * j
        k0 = q0 - WIN
        pad = max(0, -k0)
        lo = max(0, k0)
        xTc = np.zeros((C, KR), dtype=bf)
        xTc[:, pad:] = x[b, lo:q0 + QR, :].T.astype(bf)
        xt_p = np.ascontiguousarray(
            xTc.reshape(NCT, 128, KR).transpose(1, 0, 2).reshape(128, NCT * KR))
        z = x[b, lo:q0 + QR, :VEC] @ Wgate
        gate = 2.0 / (1.0 + np.exp(-z))
        veg = (ve[b, lo:q0 + QR, :].reshape(-1, NKV, HD)
               * gate[:, :, None]).reshape(-1, NKV * HD)
        vec = np.zeros((KR, NKV * HD), dtype=np.float32)
        vec[pad:] = veg
        validc = np.zeros((KR, NKV), dtype=np.float32)
        validc[pad:] = 1.0
        aux = np.concatenate([vec, validc], axis=1)  # [KR, 260]
        aux_p = np.ascontiguousarray(
            aux.reshape(NKT, 128, 260).transpose(1, 0, 2)
            .reshape(128, NKT * 260).astype(bf))
        cosc = np.zeros((KR, 32), dtype=np.float32)
        cosc[pad:] = cos2[lo:q0 + QR]
        sinc = np.zeros((KR, 32), dtype=np.float32)
        sinc[pad:] = sin2[lo:q0 + QR]
        csc = np.concatenate([cosc, sinc, sinc, cosc], axis=1)  # [KR, 128]
        csc_p = np.ascontiguousarray(
            csc.reshape(NKT, 128, 128).transpose(1, 0, 2)
            .reshape(128, NKT * 128))
        in_maps.append({
            "wkv": wkv_p, "xt": xt_p, "aux": aux_p, "csc": csc_p,
            "wq": wq_p, "wp": wp_p,
        })
    return in_maps


def kernel(x, ve, cos, sin, Wq, Wk, Wv, Wproj, Wgate, window_size, **_):
    assert int(window_size) == WIN, f"kernel hardcodes window={WIN}"
    x = np.asarray(x, dtype=np.float32)
    ve = np.asarray(ve, dtype=np.float32)
    cos = np.asarray(cos, dtype=np.float32)
    sin = np.asarray(sin, dtype=np.float32)
    in_maps = _prep_inputs(x, ve, cos, sin,
                           np.asarray(Wq, np.float32), np.asarray(Wk, np.float32),
                           np.asarray(Wv, np.float32), np.asarray(Wproj, np.float32),
                           np.asarray(Wgate, np.float32))
    nc = _get_program()
    for attempt in range(3):
        res = run_bass_kernel_spmd(nc, in_maps, list(range(N_CORES)))
        out = np.empty((B, T, C), dtype=np.float32)
        for c in range(N_CORES):
            b, j = divmod(c, N_CORES // B)
            out[b, QR * j:QR * (j + 1), :] = np.asarray(
                res.results[c]["y"]).astype(np.float32)
        if np.isfinite(out).all():
            break
    return out


if __name__ == "__main__":
    rng = np.random.default_rng(0)
    ins = {
        "x": rng.standard_normal((B, T, C), dtype=np.float32),
        "ve": rng.standard_normal((B, T, NKV * HD), dtype=np.float32),
        "cos": rng.standard_normal((1, T, 1, 32), dtype=np.float32),
        "sin": rng.standard_normal((1, T, 1, 32), dtype=np.float32),
        "Wq": rng.standard_normal((C, NH * HD), dtype=np.float32) * 0.02,
        "Wk": rng.standard_normal((C, NKV * HD), dtype=np.float32) * 0.02,
        "Wv": rng.standard_normal((C, NKV * HD), dtype=np.float32) * 0.02,
        "Wproj": rng.standard_normal((C, C), dtype=np.float32) * 0.02,
        "Wgate": rng.standard_normal((VEC, NKV), dtype=np.float32) * 0.02,
        "window_size": 1024,
    }
    y = kernel(**ins)
    print("ran, out shape", y.shape, "mean", float(np.abs(y).mean()))


# revision 25
# speedup vs baseline: 1.1184x; 1.1184x over previous
"""Trainium2 Bass kernel for nn_CausalSelfAttention_70832600646065 (v2).

Sliding-window causal GQA attention (B=2, T=2048, C=1024, NH=16, NKV=4,
HD=64, window=1024) with RoPE + RMSNorm on q/k, a value-embedding gate, and
an output projection.

Sharding: sequence-parallel over 8 cores. Core c handles batch c//4, query
rows [512*(c%4), 512*(c%4)+512) with a 1024-row key/value halo.

v2 changes vs v1 (248.7us):
  - All inputs host-packed into contiguous [128, N] tensors; priority-ordered
    DMA issue (wkv+xt first on the sync ring; wq/wp mid-phase-A on the
    scalar ring) so the first projection matmul starts ~3us in.
  - PE warmup burst at t=0 (HAM clock-gate release) covering the DMA lead-in.
  - All transposes (K, Q, Y) moved off the tensor engine onto the DMA XBAR
    (dma_start transpose=True on the HWDGE rings): kills 88 PE transposes
    and their PSUM->SBUF copies.
  - K/V projections fused into one 512-wide matmul chain per row tile.
  - Per-rt pipelined phase A (proj -> rope -> norm -> transpose per tile).
  - Attention: query heads permuted (host-side) so device pair p = heads
    (2p, 2p+1) sit in opposite PE row halves (GQA group parity) -> QK
    matmuls issued interleaved to disjoint 64-row groups run concurrently.
  - PSUM: 2x3-bank score strips + 2x1-bank AV accumulators (4 heads each)
    = 8 banks exactly; out-projection PSUM time-shares the accumulator
    slots via matching tags.
  - Softmax denominators via a validity 65th column of V (as v1); one
    batched reciprocal + one fused normalize multiply per 4 heads.
  - Output stored/DMA'd as bf16 (host casts to f32).
"""

import sys

if "/opt/trn_rl_repo" not in sys.path:
    sys.path.insert(0, "/opt/trn_rl_repo")

import numpy as np
import ml_dtypes

import concourse.bass as bass
import concourse.bacc as bacc
import concourse.mybir as mybir
import concourse.tile as tile
from concourse.bass_utils import run_bass_kernel_spmd
from concourse.masks import make_identity

F32 = mybir.dt.float32
BF16 = mybir.dt.bfloat16
AF = mybir.ActivationFunctionType
OP = mybir.AluOpType

B, T, C = 2, 2048, 1024
NH, NKV, HD = 16, 4, 64
VEC = 32
WIN = 1024
QR = 512           # query rows per core
KR = QR + WIN      # key rows per core (incl. halo)
NQT = QR // 128    # 4 query row tiles
NKT = KR // 128    # 12 key row tiles
NCT = C // 128     # 8 contraction tiles
NJB = WIN // 128 + 1  # 9 key tiles in any 128-row query tile's window
EPS = float(np.finfo(np.float32).eps)
N_CORES = 8

# device q-head order: pair p = (dh[2p], dh[2p+1]) pairs an even-group head
# with an odd-group head so the two heads of a pair use disjoint PE row halves
DH = [0, 4, 1, 5, 2, 6, 3, 7, 8, 12, 9, 13, 10, 14, 11, 15]
GDEV = [h // 4 for h in DH]          # kv group per device head


def _rope_stats(nc, pools, src_sb, cs_t, sc_t, rr_dst, ms_dst, nh):
    """src_sb: [128, nh*64] bf16 SBUF in two-major layout (x1 of all heads |
    x2 of all heads). rr_dst: same layout, rope output. ms_dst: [128, nh]
    fp32 receiving sum(rr^2) per head. cs_t: [128, 2, 32] (cos|sin) AP;
    sc_t: (sin|cos)."""
    hw = nh * 32
    tA = pools["rtmp"].tile([128, nh * HD], BF16, tag="tA", name="tA")
    tB = pools["rtmp"].tile([128, nh * HD], BF16, tag="tB", name="tB")
    sq = pools["rtmp"].tile([128, nh * HD], BF16, tag="sq", name="sq")
    s4 = src_sb.rearrange("p (two h d) -> p two h d", two=2, d=32)
    a4 = tA[:].rearrange("p (two h d) -> p two h d", two=2, d=32)
    b4 = tB[:].rearrange("p (two h d) -> p two h d", two=2, d=32)
    csb = cs_t.unsqueeze(2).broadcast_to([128, 2, nh, 32])
    scb = sc_t.unsqueeze(2).broadcast_to([128, 2, nh, 32])
    # tA = (x1*cos | x2*sin); tB = (x1*sin | x2*cos)  (split across engines)
    nc.gpsimd.tensor_tensor(a4, s4, csb, op=OP.mult)
    nc.vector.tensor_tensor(b4, s4, scb, op=OP.mult)
    # rr = (x1*cos + x2*sin | x2*cos - x1*sin)
    nc.vector.tensor_tensor(rr_dst[:, 0:hw], tA[:, 0:hw], tA[:, hw:2 * hw],
                            op=OP.add)
    nc.vector.tensor_tensor(rr_dst[:, hw:2 * hw], tB[:, hw:2 * hw],
                            tB[:, 0:hw], op=OP.subtract)
    nc.scalar.activation(sq[:], rr_dst, AF.Square)
    sq4 = sq[:].rearrange("p (two h d) -> p two h d", two=2, d=32)
    mtmp = pools["ms"].tile([128, 2 * nh], F32, tag="mtmp", name="mtmp")
    nc.vector.tensor_reduce(mtmp[:], sq4, axis=mybir.AxisListType.X, op=OP.add)
    m2 = mtmp[:].rearrange("p (two h) -> p two h", two=2)
    nc.gpsimd.tensor_tensor(ms_dst, m2[:, 0], m2[:, 1], op=OP.add)


def _rsqrt(nc, pools, ms, n, tag):
    """rsqrt(ms*(1/HD) + eps) over a [128, n] fp32 tile on DVE only."""
    nc.vector.tensor_scalar(ms, ms, 1.0 / HD, EPS, op0=OP.mult, op1=OP.add)
    sh = pools["ms"].tile([128, n], mybir.dt.int32, tag=f"sh{tag}",
                          name=f"sh{tag}")
    nc.vector.tensor_scalar(sh[:], ms.bitcast(mybir.dt.int32), 1, None,
                            op0=OP.logical_shift_right)
    nc.vector.tensor_scalar(sh[:], sh[:], -1, 0x5F3759DF, op0=OP.mult,
                            op1=OP.add)
    r0 = sh[:].bitcast(F32)
    t0 = pools["ms"].tile([128, n], F32, tag=f"t0{tag}", name=f"t0{tag}")
    for _ in range(1):
        nc.vector.tensor_tensor(t0[:], r0, r0, op=OP.mult)
        nc.vector.scalar_tensor_tensor(t0[:], ms, -0.5, t0[:],
                                       op0=OP.mult, op1=OP.mult)
        nc.vector.scalar_tensor_tensor(r0, t0[:], 1.5, r0,
                                       op0=OP.add, op1=OP.mult)
    return r0


def build_program():
    nc = bacc.Bacc("TRN2", target_bir_lowering=False, debug=False,
                   num_devices=N_CORES)

    wkv_d = nc.declare_dram_parameter("wkv", [128, NCT * 512], BF16,
                                      isOutput=False)
    xt_d = nc.declare_dram_parameter("xt", [128, NCT * KR], BF16,
                                     isOutput=False)
    aux_d = nc.declare_dram_parameter("aux", [128, NKT * 260], BF16,
                                      isOutput=False)
    csc_d = nc.declare_dram_parameter("csc", [128, NKT * 128], F32,
                                      isOutput=False)
    wq_d = nc.declare_dram_parameter("wq", [128, NCT * 1024], BF16,
                                     isOutput=False)
    wp_d = nc.declare_dram_parameter("wp", [128, NCT * 1024], BF16,
                                     isOutput=False)
    y_d = nc.declare_dram_parameter("y", [QR, C], BF16, isOutput=True)

    with tile.TileContext(nc) as tc:
        with (
            tc.tile_pool(name="wgt", bufs=1) as wgt,
            tc.tile_pool(name="persist", bufs=1) as persist,
            tc.tile_pool(name="small", bufs=1) as small,
        ):
            # ---- input DMAs, priority order on the sync ring ---------------
            wkv_all = wgt.tile([128, NCT, 512], BF16, tag="wkv", name="wkv_all")
            nc.sync.dma_start(wkv_all[:], wkv_d.ap().rearrange(
                "p (c k) -> p c k", c=NCT))
            xt_all = wgt.tile([128, NCT, KR], BF16, tag="xt", name="xt_all")
            nc.sync.dma_start(xt_all[:], xt_d.ap().rearrange(
                "p (c k) -> p c k", c=NCT))
            aux_all = wgt.tile([128, NKT, 260], BF16, tag="aux", name="aux_all")
            nc.sync.dma_start(aux_all[:], aux_d.ap().rearrange(
                "p (r k) -> p r k", r=NKT))
            csc_all = wgt.tile([128, NKT, 4, 32], F32, tag="csc",
                               name="csc_all")
            nc.sync.dma_start(csc_all[:], csc_d.ap().rearrange(
                "p (r two d) -> p r two d", r=NKT, two=4))
            wq_all = wgt.tile([128, NCT, 1024], BF16, tag="wq", name="wq_all")
            nc.sync.dma_start(wq_all[:], wq_d.ap().rearrange(
                "p (c k) -> p c k", c=NCT))
            wp_all = wgt.tile([128, NCT, 1024], BF16, tag="wp", name="wp_all")
            nc.sync.dma_start(wp_all[:], wp_d.ap().rearrange(
                "p (c k) -> p c k", c=NCT))

            xt_sb = [xt_all[:, ct] for ct in range(NCT)]
            wkv_sb = [wkv_all[:, ct] for ct in range(NCT)]
            wq_sb = [wq_all[:, ct] for ct in range(NCT)]
            wp_sb = [wp_all[:, ct] for ct in range(NCT)]
            cs_sb = [csc_all[:, rt, 0:2] for rt in range(NKT)]
            sc_sb = [csc_all[:, rt, 2:4] for rt in range(NKT)]

            ident = small.tile([128, 128], BF16, tag="ident")
            make_identity(nc, ident[:])
            # mask_lo: keep p >= f (window edge, jb==0)
            mask_lo = small.tile([128, 128], BF16, tag="mask_lo")
            nc.gpsimd.memset(mask_lo[:], 1.0)
            nc.gpsimd.affine_select(
                out=mask_lo[:], in_=mask_lo[:], compare_op=OP.is_ge, fill=0.0,
                base=0, pattern=[[-1, 128]], channel_multiplier=1,
            )
            # mask_hi: keep p <= f (causal diagonal, jb==8)
            mask_hi = small.tile([128, 128], BF16, tag="mask_hi")
            nc.gpsimd.memset(mask_hi[:], 1.0)
            nc.gpsimd.affine_select(
                out=mask_hi[:], in_=mask_hi[:], compare_op=OP.is_ge, fill=0.0,
                base=0, pattern=[[1, 128]], channel_multiplier=-1,
            )

            # persistent intermediates (single tiles; blocked along dim 1 so
            # one batched DMA-transpose per source tile fills all blocks)
            KT_all = persist.tile([128, NKV // 2, KR], BF16, tag="KT",
                                  name="KT_all")
            QT_all = persist.tile([128, NH // 2, QR], BF16, tag="QT",
                                  name="QT_all")
            Vv_sb = [persist.tile([128, NKV, HD + 1], BF16, tag=f"Vv{rt}",
                                  name=f"Vv{rt}") for rt in range(NKT)]
            YT_all = persist.tile([128, NCT, QR], BF16, tag="YT",
                                  name="YT_all")

            # ---- phase A ---------------------------------------------------
            with (
                tc.tile_pool(name="pkv", bufs=2, space="PSUM") as pkv,
                tc.tile_pool(name="pwarm", bufs=1, space="PSUM") as pwarm,
                tc.tile_pool(name="asb", bufs=3) as asb,
                tc.tile_pool(name="asm", bufs=3) as asm,
                tc.tile_pool(name="astat", bufs=1) as astat,
            ):
                pools = {"rtmp": asb, "ms": asm}

                # PE warmup: ~10us of dummy matmuls (no DMA deps) to release
                # the HAM clock gate while inputs stream in.
                wscr = small.tile([128, 512], BF16, tag="wscr", name="wscr")
                nc.gpsimd.memset(wscr[:], 0.0)
                warm = pwarm.tile([128, 512], F32, tag="warm", name="warm")
                for w in range(24):
                    nc.tensor.matmul(warm[:], ident[:], wscr[:],
                                     start=(w == 0), stop=(w == 23))

                ms_k = astat.tile([128, NKT * NKV], F32, tag="ms_k")
                ms_q = astat.tile([128, NQT * NH], F32, tag="ms_q")
                rrk_sb = [persist.tile([128, NKV * HD], BF16, tag=f"rrk{rt}",
                                       name=f"rrk{rt}") for rt in range(NKT)]
                rrq_sb = [persist.tile([128, NH * HD], BF16, tag=f"rrq{i}",
                                       name=f"rrq{i}") for i in range(NQT)]

                def kv_stats(rt):
                    rs = slice(rt * 128, (rt + 1) * 128)
                    kvp = pkv.tile([128, 512], F32, tag="kvp", name="kvp")
                    for ct in range(NCT):
                        nc.tensor.matmul(kvp[:], xt_sb[ct][:, rs], wkv_sb[ct],
                                         start=(ct == 0), stop=(ct == NCT - 1))
                    ksb = asb.tile([128, NKV * HD], BF16, tag="ksb",
                                   name="ksb")
                    nc.scalar.copy(ksb[:], kvp[:, 0:256])
                    # V = vp + ve_gated (gate precomputed on host)
                    ve3 = aux_all[:, rt, 0:256].rearrange(
                        "p (h d) -> p h d", h=NKV)
                    vp3 = kvp[:, 256:512].rearrange("p (h d) -> p h d", h=NKV)
                    nc.vector.tensor_tensor(Vv_sb[rt][:, :, 0:HD], vp3, ve3,
                                            op=OP.add)
                    nc.gpsimd.tensor_copy(
                        Vv_sb[rt][:, :, HD:HD + 1],
                        aux_all[:, rt, 256:260].unsqueeze(2))
                    _rope_stats(nc, pools, ksb[:], cs_sb[rt], sc_sb[rt],
                                rrk_sb[rt][:], ms_k[:, rt * NKV:(rt + 1) * NKV],
                                NKV)

                def kv_fin(rts):
                    # batched rsqrt over a group of row tiles, then per-tile
                    # normalize (gpsimd) + one batched DMA-transpose each
                    rk = _rsqrt(nc, pools,
                                ms_k[:, rts[0] * NKV:(rts[-1] + 1) * NKV],
                                NKV * len(rts), f"k{rts[0]}")
                    for i, rt in enumerate(rts):
                        rs = slice(rt * 128, (rt + 1) * 128)
                        kn = asb.tile([128, NKV * HD], BF16, tag="kn",
                                      name="kn")
                        kn4 = kn[:].rearrange("p (h two d) -> p two h d",
                                              two=2, d=32)
                        rr4 = rrk_sb[rt][:].rearrange(
                            "p (two h d) -> p two h d", two=2, d=32)
                        rkb = rk[:, i * NKV:(i + 1) * NKV].unsqueeze(1) \
                            .unsqueeze(3).broadcast_to([128, 2, NKV, 32])
                        nc.gpsimd.tensor_tensor(kn4, rr4, rkb, op=OP.mult)
                        nc.sync.dma_start(KT_all[:, :, rs], kn[:],
                                          transpose=True)

                def q_stats(it):
                    rt = (WIN // 128) + it
                    rs = slice(rt * 128, (rt + 1) * 128)
                    qsb = asb.tile([128, NH * HD], BF16, tag="qsb", name="qsb")
                    for half in range(2):
                        qp = pkv.tile([128, 512], F32, tag="kvp", name="qp")
                        for ct in range(NCT):
                            nc.tensor.matmul(
                                qp[:], xt_sb[ct][:, rs],
                                wq_sb[ct][:, half * 512:(half + 1) * 512],
                                start=(ct == 0), stop=(ct == NCT - 1))
                        nc.scalar.copy(qsb[:, half * 512:(half + 1) * 512],
                                       qp[:])
                    _rope_stats(nc, pools, qsb[:], cs_sb[rt], sc_sb[rt],
                                rrq_sb[it][:], ms_q[:, it * NH:(it + 1) * NH],
                                NH)

                def q_fin(its_):
                    rq = _rsqrt(nc, pools,
                                ms_q[:, its_[0] * NH:(its_[-1] + 1) * NH],
                                NH * len(its_), f"q{its_[0]}")
                    for i, it in enumerate(its_):
                        qn = asb.tile([128, NH * HD], BF16, tag="qsb",
                                      name="qn")
                        qn4 = qn[:].rearrange("p (h two d) -> p two h d",
                                              two=2, d=32)
                        rr4 = rrq_sb[it][:].rearrange(
                            "p (two h d) -> p two h d", two=2, d=32)
                        rqb = rq[:, i * NH:(i + 1) * NH].unsqueeze(1) \
                            .unsqueeze(3).broadcast_to([128, 2, NH, 32])
                        nc.gpsimd.tensor_tensor(qn4, rr4, rqb, op=OP.mult)
                        nc.sync.dma_start(
                            QT_all[:, :, it * 128:(it + 1) * 128],
                            qn[:], transpose=True)

                # KV halo tiles first (qt0's attention needs KT 0-8 + QT it0);
                # wq lands after wkv+xt so Q tiles follow the halo tiles
                for rt in range(3):
                    kv_stats(rt)
                kv_fin([0, 1, 2])
                for rt in range(3, 6):
                    kv_stats(rt)
                kv_fin([3, 4, 5])
                for rt in range(6, 9):
                    kv_stats(rt)
                kv_fin([6, 7, 8])
                q_stats(0)
                q_stats(1)
                q_fin([0, 1])
                for rt in range(9, 12):
                    kv_stats(rt)
                kv_fin([9, 10, 11])
                q_stats(2)
                q_stats(3)
                q_fin([2, 3])

            # ---- phase B: attention + fused output projection --------------
            with (
                tc.tile_pool(name="pst", bufs=1, space="PSUM") as pst,
                tc.tile_pool(name="pacc", bufs=1, space="PSUM") as pacc,
                tc.tile_pool(name="bpt", bufs=2) as bpt,
                tc.tile_pool(name="brc", bufs=4) as brc,
                tc.tile_pool(name="by", bufs=2) as by,
                tc.tile_pool(name="bob", bufs=2) as bob,
            ):
                ob_prev = None  # (qt, Y tile) awaiting out-projection

                def emit_oproj(qt_prev, half):
                    tag = "pa" if half == 0 else "pb"
                    pr = pacc.tile([128, 512], F32, tag=tag, name=f"pr{half}")
                    for ct in range(NCT):
                        nc.tensor.matmul(
                            pr[:],
                            YT_all[:, ct, qt_prev * 128:(qt_prev + 1) * 128],
                            wp_sb[ct][:, half * 512:(half + 1) * 512],
                            start=(ct == 0), stop=(ct == NCT - 1))
                    nc.vector.tensor_copy(
                        ob_prev[1][:, half * 512:(half + 1) * 512], pr[:])
                    if half == 1:
                        nc.sync.dma_start(
                            y_d.ap()[qt_prev * 128:(qt_prev + 1) * 128, :],
                            ob_prev[1][:])

                for qt in range(NQT):
                    its = slice(qt * 128, (qt + 1) * 128)
                    Y_t = by.tile([128, C], BF16, tag="Y", name="Yt")
                    acc = {}

                    def emit_av(p, pt2):
                        # AV for pair p into its batch accumulator; after the
                        # second pair of a batch: normalize + Y transposes.
                        batch, sub = divmod(p, 2)
                        tag = "pa" if (batch % 2) == 0 else "pb"
                        if sub == 0:
                            acc[batch] = pacc.tile([128, 2, 2, HD + 1], F32,
                                                   tag=tag, name=f"acc{batch}")
                        for hh in range(2):
                            g = GDEV[2 * p + hh]
                            for jb in range(NJB):
                                nc.tensor.matmul(
                                    acc[batch][:, sub, hh],
                                    pt2[:, hh, jb * 128:(jb + 1) * 128],
                                    Vv_sb[qt + jb][:, g, :],
                                    start=(jb == 0), stop=(jb == NJB - 1))
                        if sub == 1:
                            a = acc[batch]
                            rc = brc.tile([128, 4], F32, tag="rc", name="rc")
                            nc.vector.reciprocal(
                                rc[:],
                                a[:, :, :, HD].rearrange("p a b -> p (a b)"))
                            rcb = rc[:].rearrange("p (a b) -> p a b", a=2) \
                                .unsqueeze(3).broadcast_to([128, 2, 2, HD])
                            yv = Y_t[:, batch * 256:(batch + 1) * 256] \
                                .rearrange("p (a b d) -> p a b d", a=2, b=2)
                            nc.vector.tensor_tensor(yv, a[:, :, :, 0:HD], rcb,
                                                    op=OP.mult)
                            if batch == 3:
                                # one batched transpose for the whole Y tile
                                nc.sync.dma_start(YT_all[:, :, its], Y_t[:],
                                                  transpose=True)

                    prev = None  # (p, pt2) exp'd but AV not yet emitted
                    for p in range(8):
                        ktp = KT_all[:, p // 4]
                        # QK: heads (2p, 2p+1) interleaved to row halves;
                        # both strips live in one padded 6-bank PSUM tile
                        st2 = pst.tile([128, 2, NJB * 128], F32, tag="st",
                                       padded_shape=[128, 2, 1280],
                                       name="st2")
                        for jb in range(NJB):
                            jts = slice((qt + jb) * 128, (qt + jb + 1) * 128)
                            js = slice(jb * 128, (jb + 1) * 128)
                            nc.tensor.matmul(st2[:, 0, js], ktp[0:64, jts],
                                             QT_all[0:64, p, its],
                                             start=True, stop=True)
                            nc.tensor.matmul(st2[:, 1, js], ktp[64:128, jts],
                                             QT_all[64:128, p, its],
                                             start=True, stop=True)
                        # previous pair's AV fills the PE while this pair's
                        # exp runs on the scalar engine
                        if prev is not None:
                            emit_av(*prev)
                        # out-projection of the previous row tile fills the
                        # remaining PE gap (tag rotation: pa frees after
                        # normalize(batch0) ~p==3, pb after batch1 ~p==6)
                        if ob_prev is not None and p == 4:
                            emit_oproj(ob_prev[0], 0)
                        # softmax: one exp per pair (no max-subtraction
                        # needed: |s|/8 <= 8), masks batched over both heads
                        pt2 = bpt.tile([128, 2, NJB * 128], BF16, tag="pt",
                                       name="pt2")
                        nc.scalar.activation(pt2[:], st2[:], AF.Exp,
                                             scale=1.0 / np.sqrt(HD))
                        nc.vector.tensor_tensor(
                            pt2[:, :, 0:128], pt2[:, :, 0:128],
                            mask_lo[:].unsqueeze(1).broadcast_to([128, 2, 128]),
                            op=OP.mult)
                        nc.vector.tensor_tensor(
                            pt2[:, :, WIN:WIN + 128], pt2[:, :, WIN:WIN + 128],
                            mask_hi[:].unsqueeze(1).broadcast_to([128, 2, 128]),
                            op=OP.mult)
                        prev = (p, pt2)
                    emit_av(*prev)
                    if ob_prev is not None:
                        emit_oproj(ob_prev[0], 1)
                    ob_prev = (qt, bob.tile([128, C], BF16, tag="ob",
                                            name="ob"))
                emit_oproj(ob_prev[0], 0)
                emit_oproj(ob_prev[0], 1)
    nc.compile()
    return nc


_CACHED = {}


def _get_program():
    if "nc" not in _CACHED:
        _CACHED["nc"] = build_program()
    return _CACHED["nc"]


def _prep_inputs(x, ve, cos, sin, Wq, Wk, Wv, Wproj, Wgate):
    bf = ml_dtypes.bfloat16
    # K: natural head order, two-major per rope convention
    wk2 = Wk.reshape(C, NKV, 2, 32).transpose(0, 2, 1, 3).reshape(C, NKV * HD)
    # fused [Wk2 | Wv] then ct-blocked [128, NCT*512]
    wkv = np.concatenate([wk2, Wv], axis=1)  # [C, 512]
    wkv_p = np.ascontiguousarray(
        wkv.reshape(NCT, 128, 512).transpose(1, 0, 2).reshape(128, NCT * 512)
        .astype(bf))
    # Q: device head order DH, two-major
    wq2 = (Wq.reshape(C, NH, 2, 32)[:, DH].transpose(0, 2, 1, 3)
           .reshape(C, NH * HD))
    wq_p = np.ascontiguousarray(
        wq2.reshape(NCT, 128, 1024).transpose(1, 0, 2).reshape(128, NCT * 1024)
        .astype(bf))
    # Wproj rows permuted to device head order
    wp2 = Wproj.reshape(NH, HD, C)[DH].reshape(C, C)
    wp_p = np.ascontiguousarray(
        wp2.reshape(NCT, 128, 1024).transpose(1, 0, 2).reshape(128, NCT * 1024)
        .astype(bf))
    cos2 = cos[0, :, 0, :]
    sin2 = sin[0, :, 0, :]
    in_maps = []
    for c in range(N_CORES):
        b, j = divmod(c, N_CORES // B)
        q0 = QR * j
        k0 = q0 - WIN
        pad = max(0, -k0)
        lo = max(0, k0)
        xTc = np.zeros((C, KR), dtype=bf)
        xTc[:, pad:] = x[b, lo:q0 + QR, :].T.astype(bf)
        xt_p = np.ascontiguousarray(
            xTc.reshape(NCT, 128, KR).transpose(1, 0, 2).reshape(128, NCT * KR))
        z = x[b, lo:q0 + QR, :VEC] @ Wgate
        gate = 2.0 / (1.0 + np.exp(-z))
        veg = (ve[b, lo:q0 + QR, :].reshape(-1, NKV, HD)
               * gate[:, :, None]).reshape(-1, NKV * HD)
        vec = np.zeros((KR, NKV * HD), dtype=np.float32)
        vec[pad:] = veg
        validc = np.zeros((KR, NKV), dtype=np.float32)
        validc[pad:] = 1.0
        aux = np.concatenate([vec, validc], axis=1)  # [KR, 260]
        aux_p = np.ascontiguousarray(
            aux.reshape(NKT, 128, 260).transpose(1, 0, 2)
            .reshape(128, NKT * 260).astype(bf))
        cosc = np.zeros((KR, 32), dtype=np.float32)
        cosc[pad:] = cos2[lo:q0 + QR]
        sinc = np.zeros((KR, 32), dtype=np.float32)
        sinc[pad:] = sin2[lo:q0 + QR]
        csc = np.concatenate([cosc, sinc, sinc, cosc], axis=1)  # [KR, 128]
        csc_p = np.ascontiguousarray(
            csc.reshape(NKT, 128, 128).transpose(1, 0, 2)
            .reshape(128, NKT * 128))
        in_maps.append({
            "wkv": wkv_p, "xt": xt_p, "aux": aux_p, "csc": csc_p,
            "wq": wq_p, "wp": wp_p,
        })
    return in_maps


def kernel(x, ve, cos, sin, Wq, Wk, Wv, Wproj, Wgate, window_size, **_):
    assert int(window_size) == WIN, f"kernel hardcodes window={WIN}"
    x = np.asarray(x, dtype=np.float32)
    ve = np.asarray(ve, dtype=np.float32)
    cos = np.asarray(cos, dtype=np.float32)
    sin = np.asarray(sin, dtype=np.float32)
    in_maps = _prep_inputs(x, ve, cos, sin,
                           np.asarray(Wq, np.float32), np.asarray(Wk, np.float32),
                           np.asarray(Wv, np.float32), np.asarray(Wproj, np.float32),
                           np.asarray(Wgate, np.float32))
    nc = _get_program()
    for attempt in range(3):
        res = run_bass_kernel_spmd(nc, in_maps, list(range(N_CORES)))
        out = np.empty((B, T, C), dtype=np.float32)
        for c in range(N_CORES):
            b, j = divmod(c, N_CORES // B)
            out[b, QR * j:QR * (j + 1), :] = np.asarray(
                res.results[c]["y"]).astype(np.float32)
        if np.isfinite(out).all():
            break
    return out


if __name__ == "__main__":
    rng = np.random.default_rng(0)
    ins = {
        "x": rng.standard_normal((B, T, C), dtype=np.float32),
        "ve": rng.standard_normal((B, T, NKV * HD), dtype=np.float32),
        "cos": rng.standard_normal((1, T, 1, 32), dtype=np.float32),
        "sin": rng.standard_normal((1, T, 1, 32), dtype=np.float32),
        "Wq": rng.standard_normal((C, NH * HD), dtype=np.float32) * 0.02,
        "Wk": rng.standard_normal((C, NKV * HD), dtype=np.float32) * 0.02,
        "Wv": rng.standard_normal((C, NKV * HD), dtype=np.float32) * 0.02,
        "Wproj": rng.standard_normal((C, C), dtype=np.float32) * 0.02,
        "Wgate": rng.standard_normal((VEC, NKV), dtype=np.float32) * 0.02,
        "window_size": 1024,
    }
    y = kernel(**ins)
    print("ran, out shape", y.shape, "mean", float(np.abs(y).mean()))
